# revision 20
# baseline (speedup 1.0000x reference)
"""Trainium2 Bass kernel for batched per-feature cubic B-spline evaluation.

Math: the reference evaluates, per feature i, a cubic (k=3) B-spline on a
uniform grid of 48 intervals over [-1, 1] at x[b, i] in [0, 1) (so only the
24 knot intervals starting at 24 are ever active):

    y[b, i] = sum_c coef[i, c] * B_c(x[b, i])

On interval k (u = 24x - k in [0, 1)) the spline is the cubic
P_k(u) = a0 + a1 u + a2 u^2 + a3 u^3 with

    a0 = (C0 + 4 C1 + C2)/6,  a1 = (C2 - C0)/2,
    a2 = (C0 - 2 C1 + C2)/2,  a3 = (-C0 + 3 C1 - 3 C2 + C3)/6,
    Cm = coef[i, 24 + k + m].

C2-continuity gives P_k(1) = P_{k+1}(0), so the spline telescopes into a
clamp expansion that needs no per-element gather or select:

    y = c0 + sum_{k=0}^{23} t_k (lam_k + t_k (mu_k + nu_k t_k)),
    t_k = clamp(24 x - k, 0, 1),   c0 = P_0(0).

Device mapping (features on partitions, so per-feature coefficients are
per-partition [P,1] scalars), per k:
    r     = Relu(24 x - k)                       ScalarE (bias AP)
    t     = min(r, 1)                            GpSimdE
    inner = nu_k * t + mu_k                      ScalarE (scale+bias APs)
    g     = inner * t                            VectorE stt
    h     = (g + lam_k) * t                      VectorE stt
    acc  += h                                    VectorE / GpSimdE (split)
All fp32 exact. Batch sharded 8 ways; x pre-transposed on host.
"""

import numpy as np

import concourse.bacc as bacc
import concourse.mybir as mybir
from concourse.bass_utils import run_bass_kernel_spmd
from concourse.mybir import ActivationFunctionType as AFT, AluOpType as Op
from concourse.tile import TileContext

BATCH = 8192
IN_DIM = 512
GRID_NUM = 48
K_ORD = 3
N_CORES = 8
BSH = BATCH // N_CORES          # batch rows per core
P = 128                         # SBUF partitions
NFT = IN_DIM // P               # feature tiles per core
NK = 24                         # knot intervals covering x in [0, 1)
KOFF = 24                       # first global interval index used

# engine balance (per-k assignments, tunable):
# MIN_ENG[k]: 'g'=gpsimd, 'v'=vector; INNER_ENG[k]: 'a'=scalar ACT,
# 'v'=vector ts2; G_ENG[k]: 'v' or 'g'
_GP_KS = {0, 2, 5, 7, 10, 12, 14, 17, 19, 22}
MIN_ENG = ['g' if k in _GP_KS else 'v' for k in range(24)]
INNER_ENG = ['a'] * 24
G_ENG = ['g' if k in _GP_KS else 'v' for k in range(24)]
NMM = 512                       # fp32 matmul moving-operand max
NCH = BSH // NMM                # psum column chunks per feature tile

WK_BUFS = 9
IO_BUFS = 2
SKIP_MM = False
SKIP_EW = False
HALVES = 1                      # split elementwise ops into column halves

_CACHED_NC = None
LAST_RESULTS = None             # BassKernelResults from the most recent run


def _build_nc(reps=1):
    nc = bacc.Bacc("TRN2")
    xt = nc.dram_tensor("xt", [IN_DIM, BSH], mybir.dt.float32,
                        kind="ExternalInput")
    prep = nc.dram_tensor("prep", [IN_DIM, 4 * NK + 1], mybir.dt.float32,
                          kind="ExternalInput")
    ident = nc.dram_tensor("ident", [P, P], mybir.dt.float32,
                           kind="ExternalInput")
    yt = nc.dram_tensor("yt", [IN_DIM, BSH], mybir.dt.float32,
                        kind="ExternalOutput")

    with TileContext(nc) as tc:
        with tc.tile_pool(name="io", bufs=IO_BUFS) as io, \
             tc.tile_pool(name="wk", bufs=WK_BUFS) as wk, \
             tc.tile_pool(name="ev", bufs=2 * NCH) as ev, \
             tc.tile_pool(name="ps", bufs=2 * NCH, space="PSUM") as ps, \
             tc.tile_pool(name="cf", bufs=2) as cf:
            idt = cf.tile([P, P], mybir.dt.float32, tag="id")
            nc.sync.dma_start(idt[:], ident[:])
            for rep_ft in range(reps * NFT):
                ft = rep_ft % NFT
                fs = slice(ft * P, (ft + 1) * P)
                ptile = cf.tile([P, 4 * NK + 1], mybir.dt.float32, tag="p")
                nc.sync.dma_start(ptile[:], prep[fs, :])
                xtile = io.tile([P, BSH], mybir.dt.float32, tag="x")
                nc.sync.dma_start(xtile[:], xt[fs, :])

                psum = [ps.tile([P, NMM], mybir.dt.float32, tag=f"ps{c}",
                                name=f"psum{rep_ft}_{c}")
                        for c in range(NCH)] if not SKIP_MM else []

                def lam(k):
                    return ptile[:, k:k + 1]

                def mu(k):
                    return ptile[:, NK + k:NK + k + 1]

                def nu(k):
                    return ptile[:, 2 * NK + k:2 * NK + k + 1]

                c0 = ptile[:, 3 * NK:3 * NK + 1]

                def kbias(k):
                    return ptile[:, 3 * NK + 1 + k:3 * NK + 2 + k]

                for k in range(NK if not SKIP_EW else 1):
                    r = wk.tile([P, BSH], mybir.dt.float32, tag="r")
                    t = wk.tile([P, BSH], mybir.dt.float32, tag="t")
                    inner = wk.tile([P, BSH], mybir.dt.float32, tag="i")
                    g = wk.tile([P, BSH], mybir.dt.float32, tag="g")
                    h = wk.tile([P, BSH], mybir.dt.float32, tag="h")
                    g_eng = nc.gpsimd if G_ENG[k] == 'g' else nc.vector
                    min_eng = nc.gpsimd if MIN_ENG[k] == 'g' else nc.vector

                    hw = BSH // HALVES
                    for hh in range(HALVES):
                        hs = slice(hh * hw, (hh + 1) * hw)
                        nc.scalar.activation(r[:, hs], xtile[:, hs], AFT.Relu,
                                             bias=kbias(k), scale=24.0)
                        min_eng.tensor_scalar_min(t[:, hs], r[:, hs], 1.0)
                        if INNER_ENG[k] == 'a':
                            nc.scalar.activation(inner[:, hs], t[:, hs],
                                                 AFT.Identity,
                                                 bias=mu(k), scale=nu(k))
                        else:
                            nc.vector.tensor_scalar(inner[:, hs], t[:, hs],
                                                    nu(k), mu(k),
                                                    Op.mult, Op.add)
                        g_eng.tensor_tensor(g[:, hs], inner[:, hs], t[:, hs],
                                            Op.mult)
                        nc.vector.scalar_tensor_tensor(
                            h[:, hs], g[:, hs], lam(k), t[:, hs],
                            Op.add, Op.mult)
                        if not SKIP_MM:
                            for c in range(hh * (NCH // HALVES),
                                           (hh + 1) * (NCH // HALVES)):
                                cs = slice(c * NMM, (c + 1) * NMM)
                                nc.tensor.matmul(
                                    psum[c][:], idt[:], h[:, cs],
                                    start=(k == 0),
                                    stop=(k == (NK if not SKIP_EW else 1) - 1))

                # y = psum + c0
                for c in range(NCH):
                    cs = slice(c * NMM, (c + 1) * NMM)
                    yev = ev.tile([P, NMM], mybir.dt.float32, tag="y",
                                  name=f"yev{rep_ft}_{c}")
                    src_ap = xtile[:, cs] if SKIP_MM else psum[c][:]
                    nc.vector.tensor_scalar(yev[:], src_ap, c0, None,
                                            Op.add)
                    nc.sync.dma_start(yt[fs, cs], yev[:])
    nc.compile()
    return nc


def _prep_tables(coef):
    """Pack per-feature (lam, mu, nu, c0, kbias) into one (IN_DIM, 97) f32."""
    c = coef.astype(np.float64)
    C0 = c[:, KOFF:KOFF + NK]
    C1 = c[:, KOFF + 1:KOFF + 1 + NK]
    C2 = c[:, KOFF + 2:KOFF + 2 + NK]
    C3 = c[:, KOFF + 3:KOFF + 3 + NK]
    lam = (C2 - C0) / 2
    mu = (C0 - 2 * C1 + C2) / 2
    nu = (-C0 + 3 * C1 - 3 * C2 + C3) / 6
    c0 = ((C0[:, 0] + 4 * C1[:, 0] + C2[:, 0]) / 6)[:, None]
    kb = np.broadcast_to(-np.arange(NK, dtype=np.float64), (IN_DIM, NK))
    return np.concatenate([lam, mu, nu, c0, kb], axis=1).astype(np.float32)


def kernel(x, grid, coef):
    global _CACHED_NC, LAST_RESULTS
    x = np.ascontiguousarray(np.asarray(x, dtype=np.float32))
    coef = np.asarray(coef, dtype=np.float32)
    assert x.shape == (BATCH, IN_DIM) and coef.shape == (IN_DIM, GRID_NUM + K_ORD)

    prep = _prep_tables(coef)

    if _CACHED_NC is None:
        _CACHED_NC = _build_nc()
    nc = _CACHED_NC

    xT = np.ascontiguousarray(x.T)                      # (IN_DIM, BATCH)
    ident = np.eye(P, dtype=np.float32)
    in_maps = [
        {"xt": np.ascontiguousarray(xT[:, c * BSH:(c + 1) * BSH]),
         "prep": prep, "ident": ident}
        for c in range(N_CORES)
    ]
    res = run_bass_kernel_spmd(nc, in_maps, core_ids=list(range(N_CORES)))
    LAST_RESULTS = res

    y = np.empty((BATCH, IN_DIM), np.float32)
    for c in range(N_CORES):
        y[c * BSH:(c + 1) * BSH, :] = res.results[c]["yt"].T
    return y


# revision 22
# speedup vs baseline: 1.0049x; 1.0049x over previous
"""Trainium2 Bass kernel for batched per-feature cubic B-spline evaluation.

Math: the reference evaluates, per feature i, a cubic (k=3) B-spline on a
uniform grid of 48 intervals over [-1, 1] at x[b, i] in [0, 1) (so only the
24 knot intervals starting at 24 are ever active):

    y[b, i] = sum_c coef[i, c] * B_c(x[b, i])

On interval k (u = 24x - k in [0, 1)) the spline is the cubic
P_k(u) = a0 + a1 u + a2 u^2 + a3 u^3 with

    a0 = (C0 + 4 C1 + C2)/6,  a1 = (C2 - C0)/2,
    a2 = (C0 - 2 C1 + C2)/2,  a3 = (-C0 + 3 C1 - 3 C2 + C3)/6,
    Cm = coef[i, 24 + k + m].

C2-continuity gives P_k(1) = P_{k+1}(0), so the spline telescopes into a
clamp expansion that needs no per-element gather or select:

    y = c0 + sum_{k=0}^{23} t_k (lam_k + t_k (mu_k + nu_k t_k)),
    t_k = clamp(24 x - k, 0, 1),   c0 = P_0(0).

Device mapping (features on partitions, so per-feature coefficients are
per-partition [P,1] scalars), per k:
    r     = Relu(24 x - k)                       ScalarE (bias AP)
    t     = min(r, 1)                            GpSimdE
    inner = nu_k * t + mu_k                      ScalarE (scale+bias APs)
    g     = inner * t                            VectorE stt
    h     = (g + lam_k) * t                      VectorE stt
    acc  += h                                    VectorE / GpSimdE (split)
All fp32 exact. Batch sharded 8 ways; x pre-transposed on host.
"""

import numpy as np

import concourse.bacc as bacc
import concourse.mybir as mybir
from concourse.bass_utils import run_bass_kernel_spmd
from concourse.mybir import ActivationFunctionType as AFT, AluOpType as Op
from concourse.tile import TileContext

BATCH = 8192
IN_DIM = 512
GRID_NUM = 48
K_ORD = 3
N_CORES = 8
BSH = BATCH // N_CORES          # batch rows per core
P = 128                         # SBUF partitions
NFT = IN_DIM // P               # feature tiles per core
NK = 24                         # knot intervals covering x in [0, 1)
KOFF = 24                       # first global interval index used

# engine balance (per-k assignments, tunable):
# MIN_ENG[k]: 'g'=gpsimd, 'v'=vector; INNER_ENG[k]: 'a'=scalar ACT,
# 'v'=vector ts2; G_ENG[k]: 'v' or 'g'
_GP_KS = {0, 2, 5, 7, 10, 12, 14, 17, 19, 22}
MIN_ENG = ['g' if k in _GP_KS else 'v' for k in range(24)]
INNER_ENG = ['a'] * 24
G_ENG = ['g' if k in _GP_KS else 'v' for k in range(24)]
NMM = 512                       # fp32 matmul moving-operand max
NCH = BSH // NMM                # psum column chunks per feature tile

WK_BUFS = 9
IO_BUFS = 2
SKIP_MM = False
SKIP_EW = False
HALVES = 1                      # split elementwise ops into column halves

_CACHED_NC = None
LAST_RESULTS = None             # BassKernelResults from the most recent run


def _build_nc(reps=1):
    nc = bacc.Bacc("TRN2")
    xt = nc.dram_tensor("xt", [IN_DIM, BSH], mybir.dt.float32,
                        kind="ExternalInput")
    prep = nc.dram_tensor("prep", [IN_DIM, 4 * NK + 1], mybir.dt.float32,
                          kind="ExternalInput")
    ident = nc.dram_tensor("ident", [P, P], mybir.dt.float32,
                           kind="ExternalInput")
    yt = nc.dram_tensor("yt", [IN_DIM, BSH], mybir.dt.float32,
                        kind="ExternalOutput")

    with TileContext(nc) as tc:
        with tc.tile_pool(name="io", bufs=IO_BUFS) as io, \
             tc.tile_pool(name="wk", bufs=WK_BUFS) as wk, \
             tc.tile_pool(name="ev", bufs=2 * NCH) as ev, \
             tc.tile_pool(name="ps", bufs=2 * NCH, space="PSUM") as ps, \
             tc.tile_pool(name="cf", bufs=2) as cf:
            idt = cf.tile([P, P], mybir.dt.float32, tag="id")
            nc.sync.dma_start(idt[:], ident[:])
            for rep_ft in range(reps * NFT):
                ft = rep_ft % NFT
                fs = slice(ft * P, (ft + 1) * P)
                ptile = cf.tile([P, 4 * NK + 1], mybir.dt.float32, tag="p")
                nc.sync.dma_start(ptile[:], prep[fs, :])
                xtile = io.tile([P, BSH], mybir.dt.float32, tag="x")
                nc.sync.dma_start(xtile[:], xt[fs, :])

                psum = [ps.tile([P, NMM], mybir.dt.float32, tag=f"ps{c}",
                                name=f"psum{rep_ft}_{c}")
                        for c in range(NCH)] if not SKIP_MM else []

                def lam(k):
                    return ptile[:, k:k + 1]

                def mu(k):
                    return ptile[:, NK + k:NK + k + 1]

                def nu(k):
                    return ptile[:, 2 * NK + k:2 * NK + k + 1]

                c0 = ptile[:, 3 * NK:3 * NK + 1]

                def kbias(k):
                    return ptile[:, 3 * NK + 1 + k:3 * NK + 2 + k]

                for k in range(NK if not SKIP_EW else 1):
                    if k not in (0, NK - 1):
                        r = wk.tile([P, BSH], mybir.dt.float32, tag="r",
                                    name=f"r{rep_ft}_{k}")
                    t = wk.tile([P, BSH], mybir.dt.float32, tag="t")
                    inner = wk.tile([P, BSH], mybir.dt.float32, tag="i")
                    g = wk.tile([P, BSH], mybir.dt.float32, tag="g")
                    h = wk.tile([P, BSH], mybir.dt.float32, tag="h")
                    g_eng = nc.gpsimd if G_ENG[k] == 'g' else nc.vector
                    min_eng = nc.gpsimd if MIN_ENG[k] == 'g' else nc.vector

                    hw = BSH // HALVES
                    for hh in range(HALVES):
                        hs = slice(hh * hw, (hh + 1) * hw)
                        if k == 0:
                            # s >= 0: t = min(24x, 1) in one 2-slot op
                            min_eng.tensor_scalar(t[:, hs], xtile[:, hs],
                                                  24.0, 1.0, Op.mult, Op.min)
                        elif k == NK - 1:
                            # s < 24: t = relu(24x - k), min never binds
                            nc.scalar.activation(t[:, hs], xtile[:, hs],
                                                 AFT.Relu, bias=kbias(k),
                                                 scale=24.0)
                        else:
                            nc.scalar.activation(r[:, hs], xtile[:, hs],
                                                 AFT.Relu, bias=kbias(k),
                                                 scale=24.0)
                            min_eng.tensor_scalar_min(t[:, hs], r[:, hs], 1.0)
                        if INNER_ENG[k] == 'a':
                            nc.scalar.activation(inner[:, hs], t[:, hs],
                                                 AFT.Identity,
                                                 bias=mu(k), scale=nu(k))
                        else:
                            nc.vector.tensor_scalar(inner[:, hs], t[:, hs],
                                                    nu(k), mu(k),
                                                    Op.mult, Op.add)
                        g_eng.tensor_tensor(g[:, hs], inner[:, hs], t[:, hs],
                                            Op.mult)
                        nc.vector.scalar_tensor_tensor(
                            h[:, hs], g[:, hs], lam(k), t[:, hs],
                            Op.add, Op.mult)
                        if not SKIP_MM:
                            for c in range(hh * (NCH // HALVES),
                                           (hh + 1) * (NCH // HALVES)):
                                cs = slice(c * NMM, (c + 1) * NMM)
                                nc.tensor.matmul(
                                    psum[c][:], idt[:], h[:, cs],
                                    start=(k == 0),
                                    stop=(k == (NK if not SKIP_EW else 1) - 1))

                # y = psum + c0
                for c in range(NCH):
                    cs = slice(c * NMM, (c + 1) * NMM)
                    yev = ev.tile([P, NMM], mybir.dt.float32, tag="y",
                                  name=f"yev{rep_ft}_{c}")
                    src_ap = xtile[:, cs] if SKIP_MM else psum[c][:]
                    nc.scalar.activation(yev[:], src_ap, AFT.Identity,
                                         bias=c0, scale=1.0)
                    nc.sync.dma_start(yt[fs, cs], yev[:])
    nc.compile()
    return nc


def _prep_tables(coef):
    """Pack per-feature (lam, mu, nu, c0, kbias) into one (IN_DIM, 97) f32."""
    c = coef.astype(np.float64)
    C0 = c[:, KOFF:KOFF + NK]
    C1 = c[:, KOFF + 1:KOFF + 1 + NK]
    C2 = c[:, KOFF + 2:KOFF + 2 + NK]
    C3 = c[:, KOFF + 3:KOFF + 3 + NK]
    lam = (C2 - C0) / 2
    mu = (C0 - 2 * C1 + C2) / 2
    nu = (-C0 + 3 * C1 - 3 * C2 + C3) / 6
    c0 = ((C0[:, 0] + 4 * C1[:, 0] + C2[:, 0]) / 6)[:, None]
    kb = np.broadcast_to(-np.arange(NK, dtype=np.float64), (IN_DIM, NK))
    return np.concatenate([lam, mu, nu, c0, kb], axis=1).astype(np.float32)


def kernel(x, grid, coef):
    global _CACHED_NC, LAST_RESULTS
    x = np.ascontiguousarray(np.asarray(x, dtype=np.float32))
    coef = np.asarray(coef, dtype=np.float32)
    assert x.shape == (BATCH, IN_DIM) and coef.shape == (IN_DIM, GRID_NUM + K_ORD)

    prep = _prep_tables(coef)

    if _CACHED_NC is None:
        _CACHED_NC = _build_nc()
    nc = _CACHED_NC

    xT = np.ascontiguousarray(x.T)                      # (IN_DIM, BATCH)
    ident = np.eye(P, dtype=np.float32)
    in_maps = [
        {"xt": np.ascontiguousarray(xT[:, c * BSH:(c + 1) * BSH]),
         "prep": prep, "ident": ident}
        for c in range(N_CORES)
    ]
    res = run_bass_kernel_spmd(nc, in_maps, core_ids=list(range(N_CORES)))
    LAST_RESULTS = res

    y = np.empty((BATCH, IN_DIM), np.float32)
    for c in range(N_CORES):
        y[c * BSH:(c + 1) * BSH, :] = res.results[c]["yt"].T
    return y
